# revision 1
# baseline (speedup 1.0000x reference)
"""GNN message-passing kernel for Trainium2 (Bass/Tile), 8-core SPMD.

Model (from the reference):
  h0 = relu(x @ W_in.T + b_in).T            # [500, B] -> vertices 0..500
  for l in 1..7:   agg = segment_sum(w_edge * h[edge_src]) ; h_l = relu(agg)
  out = h[out_verts].T @ W_out.T + b_out    # [B, 10]

Device strategy:
  - Data-parallel over batch: 8 cores x 256 columns each.
  - The sparse per-layer aggregation is cast as a dense matmul
    agg = A_l @ h_lower, where A_l ([500 x l*500], 32 nnz/row) is built
    on the host from (edge_src, edge_dst_local, w_edge) and streamed
    from HBM in bf16.
  - Vertex space padded to 512/layer so every layer is exactly 4
    partition tiles of 128; all matmul tiling is then uniform.
  - The out_verts gather is folded into a scattered W_out on the host,
    so the output head contracts over the whole padded vertex space.
"""

import sys

try:
    import concourse  # noqa: F401  (provided by the axon site-path)
except ImportError:
    sys.path.insert(0, "/opt/trn_rl_repo")

import numpy as np
from ml_dtypes import bfloat16

# ---- problem geometry (fixed by the problem spec) ----
B = 2048            # total batch
NC = 8              # cores
BL = B // NC        # 256 batch columns per core
IN_DIM = 784
K_IN = 896          # 784 padded to 7*128
PER = 500           # vertices per layer
PAD = 512           # padded vertices per layer (4*128)
L = 8               # layers (layer 0 = input layer)
NT = 4 * L          # 32 h tiles of 128 vertices
OUT_DIM = 10
# A rows: layer l (1..7) contributes l*512 padded source rows
A_ROWS = PAD * (L * (L - 1) // 2)   # 14336
N_CHUNK = A_ROWS // PAD             # 28 chunks of 512 rows (4 k-tiles)

_PROG = None  # compiled program cache
_PROG_KEY = None  # out-head k-tile specialization the cache was built for
_LAST_IN_MAPS = None  # kept for external profiling harnesses


def _build_program(used_tiles):
    from concourse import bacc, tile
    import concourse.mybir as mybir

    f32 = mybir.dt.float32
    bf16 = mybir.dt.bfloat16
    AF = mybir.ActivationFunctionType

    n_used = len(used_tiles)
    nc = bacc.Bacc(None, target_bir_lowering=False)
    xT_d = nc.dram_tensor("xT", [128, 7, BL], bf16, kind="ExternalInput")
    win_d = nc.dram_tensor("W_inT", [4, 128, 7, 128], bf16, kind="ExternalInput")
    bin_d = nc.dram_tensor("b_inP", [128, 4], f32, kind="ExternalInput")
    a_d = nc.dram_tensor("A", [N_CHUNK, 128, 4, PAD], bf16, kind="ExternalInput")
    wout_d = nc.dram_tensor(
        "W_outT", [128, n_used, OUT_DIM], bf16, kind="ExternalInput"
    )
    bout_d = nc.dram_tensor("b_outP", [OUT_DIM, 1], f32, kind="ExternalInput")
    out_d = nc.dram_tensor("out", [OUT_DIM, BL], f32, kind="ExternalOutput")

    with tile.TileContext(nc) as tc:
        with (
            tc.tile_pool(name="const", bufs=1) as cpool,
            tc.tile_pool(name="hbuf", bufs=1) as hpool,
            tc.tile_pool(name="astream", bufs=8) as apool,
            tc.tile_pool(name="ps", bufs=6, space="PSUM") as ppool,
            tc.tile_pool(name="pso", bufs=1, space="PSUM") as opool,
            tc.tile_pool(name="outs", bufs=1) as spool,
        ):
            # First-matmul critical path: win0 and the first xT k-tiles go
            # out first; the rest of xT/weights and the A stream queue behind.
            xt_s = cpool.tile([128, 7, BL], bf16)
            win_tiles = []
            for m in range(4):
                wt = cpool.tile([128, 7, 128], bf16, name=f"win{m}")
                win_tiles.append(wt)
            nc.scalar.dma_start(win_tiles[0][:], win_d[0])
            nc.sync.dma_start(xt_s[:, 0:2, :], xT_d[:, 0:2, :])
            nc.sync.dma_start(xt_s[:, 2:7, :], xT_d[:, 2:7, :])
            for m in range(1, 4):
                nc.scalar.dma_start(win_tiles[m][:], win_d[m])
            bin_s = cpool.tile([128, 4], f32)
            wout_s = cpool.tile([128, n_used, OUT_DIM], bf16)
            bout_s = cpool.tile([OUT_DIM, 1], f32)
            nc.scalar.dma_start(bin_s[:], bin_d[:])
            nc.scalar.dma_start(wout_s[:], wout_d[:])
            nc.scalar.dma_start(bout_s[:], bout_d[:])

            h = hpool.tile([128, NT, BL], bf16)

            # ---- input layer: h[0:4] = relu(W_in.T.T @ xT + b_in) ----
            for m in range(4):
                ps = ppool.tile([128, BL], f32)
                for kt in range(7):
                    nc.tensor.matmul(
                        ps[:],
                        win_tiles[m][:, kt, :],
                        xt_s[:, kt, :],
                        start=(kt == 0),
                        stop=(kt == 6),
                    )
                nc.scalar.activation(
                    h[:, m, :], ps[:], AF.Relu, bias=bin_s[:, m:m + 1]
                )

            # ---- hidden layers: h[4l..4l+4] = relu(A_l @ h[0:4l]) ----
            chunk = 0
            for l in range(1, L):
                nkt = 4 * l
                a_tiles = []
                for c in range(l):
                    at = apool.tile([128, 4, PAD], bf16, tag="achunk", name="at")
                    nc.sync.dma_start(at[:], a_d[chunk])
                    a_tiles.append(at)
                    chunk += 1
                pls = [
                    ppool.tile([128, BL], f32, tag="ps", name=f"pl{m}")
                    for m in range(4)
                ]
                for kt in range(nkt):
                    a_s = a_tiles[kt // 4]
                    for m in range(4):
                        nc.tensor.matmul(
                            pls[m][:],
                            a_s[:, kt % 4, m * 128:(m + 1) * 128],
                            h[:, kt, :],
                            start=(kt == 0),
                            stop=(kt == nkt - 1),
                        )
                for m in range(4):
                    nc.scalar.activation(
                        h[:, 4 * l + m, :], pls[m][:], AF.Relu, bias=0.0
                    )

            # ---- output head: out = W_outT.T @ h + b_out ----
            # contracts only over the k-tiles out_verts actually touches
            pso = opool.tile([OUT_DIM, BL], f32)
            for i, kt in enumerate(used_tiles):
                nc.tensor.matmul(
                    pso[:],
                    wout_s[:, i, :],
                    h[:, kt, :],
                    start=(i == 0),
                    stop=(i == n_used - 1),
                )
            out_s = spool.tile([OUT_DIM, BL], f32)
            nc.scalar.activation(out_s[:], pso[:], AF.Identity, bias=bout_s[:])
            nc.sync.dma_start(out_d[:], out_s[:])

    nc.compile()
    return nc


def _pack_ptiles(arr2d, n_tiles):
    """[n_tiles*128, F] row-major -> [128, n_tiles, F] partition-major."""
    f = arr2d.shape[1]
    return np.ascontiguousarray(
        arr2d.reshape(n_tiles, 128, f).transpose(1, 0, 2)
    )


def kernel(**inputs):
    global _PROG
    x = np.asarray(inputs["x"], np.float32)
    W_in = np.asarray(inputs["W_in"], np.float32)
    b_in = np.asarray(inputs["b_in"], np.float32)
    w_edge = np.asarray(inputs["w_edge"], np.float32)
    W_out = np.asarray(inputs["W_out"], np.float32)
    b_out = np.asarray(inputs["b_out"], np.float32)
    edge_src = np.asarray(inputs["edge_src"]).astype(np.int64)
    edge_dst = np.asarray(inputs["edge_dst_local"]).astype(np.int64)
    offsets = np.asarray(inputs["edge_offsets"]).astype(np.int64)
    out_verts = np.asarray(inputs["out_verts"]).astype(np.int64)

    # ---- host-side packing ----
    # A: per-layer dense adjacency, padded 512/layer, bf16, chunk-major
    A = np.zeros((A_ROWS, PAD), np.float32)
    base = 0
    for l in range(1, L):
        s, e = int(offsets[l - 1]), int(offsets[l])
        rows = base + (edge_src[s:e] // PER) * PAD + (edge_src[s:e] % PER)
        np.add.at(A, (rows, edge_dst[s:e]), w_edge[s:e])
        base += l * PAD
    A_re = np.ascontiguousarray(
        A.reshape(N_CHUNK, 4, 128, PAD).transpose(0, 2, 1, 3)
    ).astype(bfloat16)

    winT = np.zeros((K_IN, PAD), np.float32)
    winT[:IN_DIM, :PER] = W_in.T
    # [896, 512] -> [4 m-tiles][128, 7 k-tiles, 128]
    winT_re = np.ascontiguousarray(
        _pack_ptiles(winT, 7).reshape(128, 7, 4, 128).transpose(2, 0, 1, 3)
    ).astype(bfloat16)

    binP = np.zeros((PAD,), np.float32)
    binP[:PER] = b_in
    binP_re = np.ascontiguousarray(binP.reshape(4, 128).T)

    woutT = np.zeros((NT * 128, OUT_DIM), np.float32)
    pad_idx = (out_verts // PER) * PAD + (out_verts % PER)
    woutT[pad_idx, :] = W_out.T
    used_tiles = tuple(sorted(set(int(t) for t in pad_idx // 128)))
    woutT_re = np.ascontiguousarray(
        _pack_ptiles(woutT, NT)[:, list(used_tiles), :]
    ).astype(bfloat16)

    boutP = np.ascontiguousarray(b_out.reshape(OUT_DIM, 1))

    shared = {
        "W_inT": winT_re,
        "b_inP": binP_re,
        "A": A_re,
        "W_outT": woutT_re,
        "b_outP": boutP,
    }
    in_maps = []
    for c in range(NC):
        xT = np.zeros((K_IN, BL), np.float32)
        xT[:IN_DIM, :] = x[c * BL:(c + 1) * BL, :].T
        in_maps.append({"xT": _pack_ptiles(xT, 7).astype(bfloat16), **shared})

    from concourse.bass_utils import run_bass_kernel_spmd

    global _LAST_IN_MAPS, _PROG, _PROG_KEY
    _LAST_IN_MAPS = in_maps
    if _PROG is None or _PROG_KEY != used_tiles:
        _PROG = _build_program(used_tiles)
        _PROG_KEY = used_tiles
    res = run_bass_kernel_spmd(_PROG, in_maps, list(range(NC)))
    out = np.concatenate(
        [np.asarray(res.results[c]["out"], np.float32).T for c in range(NC)], axis=0
    )
    return np.ascontiguousarray(out)



# revision 5
# speedup vs baseline: 1.0487x; 1.0487x over previous
"""GNN message-passing kernel for Trainium2 (Bass/Tile), 8-core SPMD.

Model (from the reference):
  h0 = relu(x @ W_in.T + b_in).T            # [500, B] -> vertices 0..500
  for l in 1..7:   agg = segment_sum(w_edge * h[edge_src]) ; h_l = relu(agg)
  out = h[out_verts].T @ W_out.T + b_out    # [B, 10]

Device strategy:
  - Data-parallel over batch: 8 cores x 256 columns each.
  - The sparse per-layer aggregation is cast as a dense matmul
    agg = A_l @ h_lower, where A_l ([500 x l*500], 32 nnz/row) is built
    on the host from (edge_src, edge_dst_local, w_edge) and streamed
    from HBM in bf16.
  - Vertex space padded to 512/layer so every layer is exactly 4
    partition tiles of 128; all matmul tiling is then uniform.
  - out_verts handling: highway vertices (out_verts below the last
    layer) are forwarded into unused pad rows of layer 7 via 1.0
    pass-through entries in A_7 (relu-idempotent since h >= 0), and
    b_out rides a constant-1 pad vertex (seeded through b_in's pad
    bias).  The output head then contracts only the 4 last-layer
    tiles, batch-major with n=10 moving columns, and the result DMAs
    out directly as [batch, 10].

Schedule notes (tuned against the TimelineSim cost model):
  - Warmup dummy matmuls keep the PE busy from t~0 so the p-state ramp
    (0.65/1.2 GHz until 3us continuously busy) finishes before real
    matmuls start, and they plug DMA-arrival gaps in the input layer.
  - Input layer is kt-major so it consumes W_in k-slabs in DMA arrival
    order; A chunk 0 is queued right behind them.
  - Small constant DMAs (bias, W_out head) issue from the gpsimd queue
    (SWDGE) so they stay off the serialized HWDGE generator.
  - Layer 7 ends m-major so the four PSUM groups stop staggered, and
    their relus run on scalar/vector/gpsimd in parallel to shorten the
    tail into the output head.
"""

import sys

try:
    import concourse  # noqa: F401  (provided by the axon site-path)
except ImportError:
    sys.path.insert(0, "/opt/trn_rl_repo")

import numpy as np
from ml_dtypes import bfloat16

# ---- problem geometry (fixed by the problem spec) ----
B = 2048            # total batch
NC = 8              # cores
BL = B // NC        # 256 batch columns per core
IN_DIM = 784
K_IN = 896          # 784 padded to 7*128
PER = 500           # vertices per layer
PAD = 512           # padded vertices per layer (4*128)
L = 8               # layers (layer 0 = input layer)
NT = 4 * L          # 32 h tiles of 128 vertices
OUT_DIM = 10
# A rows: layer l (1..7) contributes l*512 padded source rows
A_ROWS = PAD * (L * (L - 1) // 2)   # 14336
N_CHUNK = A_ROWS // PAD             # 28 chunks of 512 rows (4 k-tiles)
MAX_HW = 11          # highway vertices foldable into layer-7 pad rows

# schedule tuning knobs (dummy warmup matmuls, n=64 columns each)
N_DUMMY_HEAD = 72    # before the first input matmul
N_DUMMY_KT0 = 16     # between input kt0 and kt1 (W_in slab arrival gap)

_PROG = None  # compiled program cache
_LAST_IN_MAPS = None  # kept for external profiling harnesses


def _build_program():
    from concourse import bacc, tile
    import concourse.mybir as mybir

    f32 = mybir.dt.float32
    bf16 = mybir.dt.bfloat16
    AF = mybir.ActivationFunctionType

    nc = bacc.Bacc(None, target_bir_lowering=False)
    xT_d = nc.dram_tensor("xT", [128, 7, BL], bf16, kind="ExternalInput")
    win_d = nc.dram_tensor("W_inT", [128, 7, 4, 128], bf16, kind="ExternalInput")
    bin_d = nc.dram_tensor("b_inP", [128, 4], f32, kind="ExternalInput")
    a_d = nc.dram_tensor("A", [N_CHUNK, 128, 4, PAD], bf16, kind="ExternalInput")
    wout_d = nc.dram_tensor("W_outP", [128, 4, OUT_DIM], bf16, kind="ExternalInput")
    out_d = nc.dram_tensor("out", [128, 2, OUT_DIM], f32, kind="ExternalOutput")

    with tile.TileContext(nc) as tc:
        with (
            tc.tile_pool(name="const", bufs=1) as cpool,
            tc.tile_pool(name="hbuf", bufs=1) as hpool,
            tc.tile_pool(name="astream", bufs=8) as apool,
            tc.tile_pool(name="ps", bufs=8, space="PSUM") as ppool,
            tc.tile_pool(name="outs", bufs=1) as spool,
        ):
            # ---- input DMAs: W_in kt0 and xT lead, A chunk 0 close behind
            win_s = cpool.tile([128, 7, 4, 128], bf16)
            xt_s = cpool.tile([128, 7, BL], bf16)
            nc.sync.dma_start(win_s[:, 0:1], win_d[:, 0:1])
            nc.sync.dma_start(xt_s[:], xT_d[:])
            nc.sync.dma_start(win_s[:, 1:4], win_d[:, 1:4])
            nc.sync.dma_start(win_s[:, 4:7], win_d[:, 4:7])
            # small constants via SWDGE (gpsimd) to stay off HWDGE
            bin_s = cpool.tile([128, 4], f32)
            wout_s = cpool.tile([128, 4, OUT_DIM], bf16)
            nc.gpsimd.dma_start(bin_s[:], bin_d[:])
            nc.gpsimd.dma_start(wout_s[:], wout_d[:])

            # ---- PE warmup: keep the engine busy from t~0 so the
            # p-state ramp completes before the first real matmul.
            scratch = cpool.tile([128, 128], bf16)
            nc.vector.memset(scratch[:], 0.0)
            psd = ppool.tile([128, 64], f32, tag="ps", name="psd")
            for _ in range(N_DUMMY_HEAD):
                nc.tensor.matmul(
                    psd[:], scratch[:], scratch[:, 0:64],
                    start=True, stop=True, skip_group_check=True,
                )

            h = hpool.tile([128, NT, BL], bf16)

            # ---- input layer (kt-major): h[0:4] = relu(W_in.T.T @ xT + b)
            psin = [ppool.tile([128, BL], f32, tag="ps", name=f"pi{m}")
                    for m in range(4)]
            for kt in range(7):
                for m in range(4):
                    nc.tensor.matmul(
                        psin[m][:],
                        win_s[:, kt, m, :],
                        xt_s[:, kt, :],
                        start=(kt == 0),
                        stop=(kt == 6),
                    )
                if kt == 0:
                    for _ in range(N_DUMMY_KT0):
                        nc.tensor.matmul(
                            psd[:], scratch[:], scratch[:, 0:64],
                            start=True, stop=True, skip_group_check=True,
                        )
            for m in range(4):
                nc.scalar.activation(
                    h[:, m, :], psin[m][:], AF.Relu, bias=bin_s[:, m:m + 1]
                )

            # ---- hidden layers: h[4l..4l+4] = relu(A_l @ h[0:4l]) ----
            chunk = 0
            for l in range(1, L):
                nkt = 4 * l
                a_tiles = []
                for c in range(l):
                    at = apool.tile([128, 4, PAD], bf16, tag="achunk", name="at")
                    nc.sync.dma_start(at[:], a_d[chunk])
                    a_tiles.append(at)
                    chunk += 1
                pls = [
                    ppool.tile([128, BL], f32, tag="ps", name=f"pl{m}")
                    for m in range(4)
                ]
                kt_major = nkt if l < L - 1 else nkt - 4
                for kt in range(kt_major):
                    a_s = a_tiles[kt // 4]
                    for m in range(4):
                        nc.tensor.matmul(
                            pls[m][:],
                            a_s[:, kt % 4, m * 128:(m + 1) * 128],
                            h[:, kt, :],
                            start=(kt == 0),
                            stop=(l < L - 1 and kt == nkt - 1),
                        )
                if l < L - 1:
                    for m in range(4):
                        nc.scalar.activation(
                            h[:, 4 * l + m, :], pls[m][:], AF.Relu, bias=0.0
                        )
                else:
                    # last layer: m-major tail staggers the PSUM stops so
                    # the four relus (on three engines) pipeline into the
                    # output head.
                    a_s = a_tiles[l - 1]
                    for m in range(4):
                        for kt in range(nkt - 4, nkt):
                            nc.tensor.matmul(
                                pls[m][:],
                                a_s[:, kt % 4, m * 128:(m + 1) * 128],
                                h[:, kt, :],
                                start=False,
                                stop=(kt == nkt - 1),
                            )
                    nc.scalar.activation(
                        h[:, 4 * l + 0, :], pls[0][:], AF.Relu, bias=0.0)
                    nc.vector.tensor_scalar_max(h[:, 4 * l + 1, :], pls[1][:], 0.0)
                    nc.gpsimd.tensor_scalar_max(h[:, 4 * l + 2, :], pls[2][:], 0.0)
                    nc.scalar.activation(
                        h[:, 4 * l + 3, :], pls[3][:], AF.Relu, bias=0.0)

            # ---- output head: outT[b, j] = sum_v h7[v, b] * W_outP[v, j]
            # batch-major, n=10 moving columns; bias folded into W_outP.
            pso = ppool.tile([128, 2, OUT_DIM], f32, tag="ps", name="pso")
            for t in range(4):
                for half in range(2):
                    nc.tensor.matmul(
                        pso[:, half, :],
                        h[:, 28 + t, half * 128:(half + 1) * 128],
                        wout_s[:, t, :],
                        start=(t == 0),
                        stop=(t == 3),
                    )
            out_s = spool.tile([128, 2, OUT_DIM], f32)
            nc.scalar.activation(out_s[:], pso[:], AF.Identity, bias=0.0)
            nc.gpsimd.dma_start(out_d[:], out_s[:])

    nc.compile()
    return nc


def _pack_ptiles(arr2d, n_tiles):
    """[n_tiles*128, F] row-major -> [128, n_tiles, F] partition-major."""
    f = arr2d.shape[1]
    return np.ascontiguousarray(
        arr2d.reshape(n_tiles, 128, f).transpose(1, 0, 2)
    )


def kernel(**inputs):
    x = np.asarray(inputs["x"], np.float32)
    W_in = np.asarray(inputs["W_in"], np.float32)
    b_in = np.asarray(inputs["b_in"], np.float32)
    w_edge = np.asarray(inputs["w_edge"], np.float32)
    W_out = np.asarray(inputs["W_out"], np.float32)
    b_out = np.asarray(inputs["b_out"], np.float32)
    edge_src = np.asarray(inputs["edge_src"]).astype(np.int64)
    edge_dst = np.asarray(inputs["edge_dst_local"]).astype(np.int64)
    offsets = np.asarray(inputs["edge_offsets"]).astype(np.int64)
    out_verts = np.asarray(inputs["out_verts"]).astype(np.int64)

    # ---- host-side packing ----
    # A: per-layer dense adjacency, padded 512/layer, bf16, chunk-major
    A = np.zeros((A_ROWS, PAD), np.float32)
    base = 0
    for l in range(1, L):
        s, e = int(offsets[l - 1]), int(offsets[l])
        rows = base + (edge_src[s:e] // PER) * PAD + (edge_src[s:e] % PER)
        np.add.at(A, (rows, edge_dst[s:e]), w_edge[s:e])
        base += l * PAD

    # Fold out_verts into layer 7: highway vertices (ids < (L-1)*PER)
    # pass through to pad dsts 500..500+n_hw-1; pad dst 511 carries a
    # constant 1.0 sourced from layer-0 pad vertex 511 (bias row).
    base7 = PAD * ((L - 1) * (L - 2) // 2)          # 10752
    hw_verts = out_verts[out_verts < (L - 1) * PER]
    n_hw = len(hw_verts)
    assert n_hw <= MAX_HW, n_hw
    assert np.array_equal(
        out_verts[n_hw:], np.arange((L - 1) * PER, L * PER)
    ), "out_verts tail must be the full last layer"
    for i, v in enumerate(hw_verts):
        p_v = (v // PER) * PAD + (v % PER)
        A[base7 + p_v, PER + i] = 1.0
    A[base7 + 511, 511] = 1.0                        # ones pass-through
    A_re = np.ascontiguousarray(
        A.reshape(N_CHUNK, 4, 128, PAD).transpose(0, 2, 1, 3)
    ).astype(bfloat16)

    winT = np.zeros((K_IN, PAD), np.float32)
    winT[:IN_DIM, :PER] = W_in.T
    # [896, 512] -> [128, 7 kt, 4 m, 128]
    winT_re = np.ascontiguousarray(
        _pack_ptiles(winT, 7).reshape(128, 7, 4, 128)
    ).astype(bfloat16)

    binP = np.zeros((PAD,), np.float32)
    binP[:PER] = b_in
    binP[511] = 1.0                                  # constant-1 pad vertex
    binP_re = np.ascontiguousarray(binP.reshape(4, 128).T)

    # Output head over layer-7 padded positions: q<500 -> last-layer
    # vertex, q=500+i -> highway i, q=511 -> bias row.
    woutP = np.zeros((PAD, OUT_DIM), np.float32)
    woutP[:PER, :] = W_out[:, n_hw:].T
    woutP[PER:PER + n_hw, :] = W_out[:, :n_hw].T
    woutP[511, :] = b_out
    woutP_re = np.ascontiguousarray(
        woutP.reshape(4, 128, OUT_DIM).transpose(1, 0, 2)
    ).astype(bfloat16)

    shared = {
        "W_inT": winT_re,
        "b_inP": binP_re,
        "A": A_re,
        "W_outP": woutP_re,
    }
    in_maps = []
    for c in range(NC):
        xT = np.zeros((K_IN, BL), np.float32)
        xT[:IN_DIM, :] = x[c * BL:(c + 1) * BL, :].T
        in_maps.append({"xT": _pack_ptiles(xT, 7).astype(bfloat16), **shared})

    from concourse.bass_utils import run_bass_kernel_spmd

    global _LAST_IN_MAPS, _PROG
    _LAST_IN_MAPS = in_maps
    if _PROG is None:
        _PROG = _build_program()
    res = run_bass_kernel_spmd(_PROG, in_maps, list(range(NC)))
    # out[c] is [128, 2, 10]: partition p, half hh -> batch hh*128+p
    out = np.concatenate(
        [
            np.asarray(res.results[c]["out"], np.float32)
            .transpose(1, 0, 2).reshape(BL, OUT_DIM)
            for c in range(NC)
        ],
        axis=0,
    )
    return np.ascontiguousarray(out)
